# revision 14
# baseline (speedup 1.0000x reference)
"""Trainium2 Bass kernel for nn_AdaptiveMask (dense MLP over upper-triangle gather).

Computation (reference):
    x_flat = x[:, iu0, iu1]                      # [B, M] strict-upper-tri gather
    h = relu(x_flat @ w1 + b1)                   # [B, H]
    m = sigmoid(h @ w2 + b2)                     # [B, M]
    W = scatter_sym(m); out = W * x              # [B, C, C]
    returns (out, m)

Strategy (8 NeuronCores):
  - GEMM1 is tensor-parallel over the hidden dim: core c holds w1[:, cols_c]
    and computes h_c = relu(x_flat @ w1_c) for the full batch, in TWO hidden
    halves so the first half's activations AllGather while the second half
    computes (the gather hides under compute).
  - The transposed activations hT ([128, 640] bf16 per half, 0.16 MB) are
    AllGathered so every core holds the full hT; GEMM2 is tensor-parallel
    over the OUTPUT columns — each core computes exact (unsummed)
    y[:, cols_c] = h @ w2[:, cols_c]: no output reduction, no y bounce.
  - All matmuls in bf16 (fp32 PSUM accumulation); epilogue in fp32 straight
    from PSUM (sigmoid on ScalarE, gating multiplies on VectorE).
  - Weights/x are pre-permuted on host into per-partition-contiguous layouts
    so every weight DMA is a plain 2D transfer at line rate.
  - DMA ring discipline: sync + scalar HWDGE rings carry only the weight
    streams (FIFO rings — anything else ahead of weights starves the PE);
    gpsimd (SWDGE) carries xT/xf/xtf, the collectives, and gather loads.
  - Biases are folded into the GEMMs (ones-column in x_flat picks up b1; a
    bias hidden-unit in core 7's hidden block injects b2, shared to all
    cores by the AllGather).
  - Host does only layout (triangle gather/scatter, pad, shard, permute,
    cast); every FLOP of the reference runs on device.
"""

import numpy as np
import ml_dtypes

import concourse.bass as bass
import concourse.bacc as bacc
import concourse.tile as tile
from concourse import mybir
from concourse import bass_utils

# ---- problem constants (must match reference.py's setup_inputs) ----
B, NCH = 128, 200
M, H = 19900, 9950
NCORES = 8
BL = B // NCORES

K1, KT1 = 19968, 156  # GEMM1 contraction (19900 data + 1 bias row + pad), k-tiles
HC = 1280             # per-core hidden slots
HHALF = HC // 2       # hidden half (640)
HTOT, KT2 = 10240, 80 # global padded hidden, GEMM2 k-tiles
H_PER = [1244] * 7 + [1242]
H_START = [0, 1244, 2488, 3732, 4976, 6220, 7464, 8708]
BIAS_SLOT = 1242      # block-7 local hidden slot for the b2 bias unit

MC = 2560             # per-core output columns (5 n-chunks of 512)
MC_REAL = [2560] * 7 + [19900 - 7 * 2560]  # real cols per core (core 7: 1980)
NCHK = 5              # n-chunks per core
KQ = 20               # k-tiles per w2 DMA quarter (4 quarters of 80)

W1_CHUNK = 8          # k-tiles per w1 DMA (1.3 MB bf16 per hidden half)

CDT = mybir.dt.bfloat16
NP_CDT = ml_dtypes.bfloat16
F32 = mybir.dt.float32

_IU = np.triu_indices(NCH, k=1)


def build_nc():
    nc = bacc.Bacc("TRN2", target_bir_lowering=False, debug=False, num_devices=NCORES)

    # host-permuted layouts: per-partition-contiguous (see prep_in_maps)
    xT = nc.dram_tensor("xT", [128, K1], CDT, kind="ExternalInput")
    w1 = nc.dram_tensor("w1", [128, KT1 * HC], CDT, kind="ExternalInput")
    w2 = nc.dram_tensor("w2", [128, KT2 * MC], CDT, kind="ExternalInput")
    xf = nc.dram_tensor("xf", [B, MC], F32, kind="ExternalInput")
    xtf = nc.dram_tensor("xtf", [B, MC], F32, kind="ExternalInput")
    om = nc.dram_tensor("om", [B, MC], F32, kind="ExternalOutput")
    ou = nc.dram_tensor("ou", [B, MC], F32, kind="ExternalOutput")
    ol = nc.dram_tensor("ol", [B, MC], F32, kind="ExternalOutput")

    with tile.TileContext(nc) as tc:
        with (
            tc.tile_pool(name="const", bufs=1) as constp,
            tc.tile_pool(name="hbuf", bufs=1) as hp,
            tc.tile_pool(name="dramc", bufs=1, space="DRAM") as dramc,
        ):
            ident_dram = nc.inline_tensor(np.eye(128, dtype=NP_CDT), name="ident")
            ident = constp.tile([128, 128], CDT)
            nc.scalar.dma_start(ident[:], ident_dram[:])

            # x_flat^T resident in SBUF (gpsimd ring: keeps HWDGE rings clear)
            xT_sb = constp.tile([128, K1], CDT)
            nc.gpsimd.dma_start(xT_sb[:], xT[:])

            h_sb = hp.tile([128, HC], CDT)      # [batch, hidden_local]
            hT_sb = hp.tile([128, HC], CDT)     # [hidden_local, batch], 10 k-tiles
            hTf_sb = hp.tile([128, HTOT], CDT)  # full gathered hT, 80 k-tiles

            # warm up the collective path while GEMM1 runs: the first ncfw
            # triggers pay a large init cost (measured 132 us cold, ~18 us warm)
            cc_wi = dramc.tile([128, 128], CDT, name="cc_wi")
            for i in range(2):
                cc_wo = dramc.tile(
                    [NCORES * 128, 128], CDT, name=f"cc_wo{i}", addr_space="Shared"
                )
                nc.gpsimd.collective_compute(
                    "AllGather",
                    mybir.AluOpType.bypass,
                    replica_groups=[list(range(NCORES))],
                    ins=[cc_wi[:].opt()],
                    outs=[cc_wo[:].opt()],
                )

            hT_bounce = [
                dramc.tile([128, HHALF], CDT, name=f"hT_bounce{h}") for h in range(2)
            ]
            hT_all = [
                dramc.tile(
                    [NCORES * 128, HHALF], CDT, name=f"hT_all{h}", addr_space="Shared"
                )
                for h in range(2)
            ]

            # ---- GEMM1: h = relu(x_flat @ w1_c + b1_c), two hidden halves ----
            with (
                tc.tile_pool(name="w1p", bufs=4) as w1p,
                tc.tile_pool(name="ps1", bufs=2, space="PSUM") as ps1,
                tc.tile_pool(name="pst", bufs=2, space="PSUM") as pst,
            ):
                n_chunks1 = [(0, 512), (512, 640)]
                n_w1_chunks = (KT1 + W1_CHUNK - 1) // W1_CHUNK
                for half in range(2):
                    ph = [
                        ps1.tile([128, n1 - n0], F32, tag=f"ph{i}", name=f"ph{i}_{half}")
                        for i, (n0, n1) in enumerate(n_chunks1)
                    ]
                    hbase = half * KT1 * HHALF
                    for c in range(n_w1_chunks):
                        kc = min(W1_CHUNK, KT1 - c * W1_CHUNK)
                        c0 = hbase + c * W1_CHUNK * HHALF
                        w1t = w1p.tile(
                            [128, kc * HHALF], CDT, tag="w1t", name=f"w1t{half}_{c}"
                        )
                        eng = nc.sync if c % 2 == 0 else nc.scalar
                        eng.dma_start(w1t[:], w1[:, c0 : c0 + kc * HHALF])
                        for j in range(kc):
                            k = c * W1_CHUNK + j
                            for i, (n0, n1) in enumerate(n_chunks1):
                                nc.tensor.matmul(
                                    ph[i][:, :],
                                    xT_sb[:, k * 128 : (k + 1) * 128],
                                    w1t[:, j * HHALF + n0 : j * HHALF + n1],
                                    start=(k == 0),
                                    stop=(k == KT1 - 1),
                                )
                    for i, (n0, n1) in enumerate(n_chunks1):
                        nc.scalar.activation(
                            h_sb[:, half * HHALF + n0 : half * HHALF + n1],
                            ph[i][:, :],
                            mybir.ActivationFunctionType.Relu,
                        )
                    # transpose this half's h -> hT via PE
                    for j in range(HHALF // 128):
                        pt = pst.tile([128, 128], CDT, tag="pt", name=f"pt{half}_{j}")
                        nc.tensor.transpose(
                            pt[:],
                            h_sb[:, half * HHALF + j * 128 : half * HHALF + (j + 1) * 128],
                            ident[:],
                        )
                        nc.vector.tensor_copy(
                            hT_sb[:, half * HHALF + j * 128 : half * HHALF + (j + 1) * 128],
                            pt[:],
                        )
                    # AllGather this half (first half hides under second half's GEMM)
                    nc.gpsimd.dma_start(
                        hT_bounce[half][:], hT_sb[:, half * HHALF : (half + 1) * HHALF]
                    )
                    nc.gpsimd.collective_compute(
                        "AllGather",
                        mybir.AluOpType.bypass,
                        replica_groups=[list(range(NCORES))],
                        ins=[hT_bounce[half][:].opt()],
                        outs=[hT_all[half][:].opt()],
                    )
                    for c in range(NCORES):
                        nc.gpsimd.dma_start(
                            hTf_sb[
                                :,
                                c * HC + half * HHALF : c * HC + (half + 1) * HHALF,
                            ],
                            hT_all[half][c * 128 : (c + 1) * 128, :],
                        )

            # ---- GEMM2 (exact column shard, no reduce) + fused epilogue ----
            with (
                tc.tile_pool(name="w2p", bufs=4) as w2p,
                tc.tile_pool(name="ps2", bufs=2, space="PSUM") as ps2,
                tc.tile_pool(name="ep", bufs=2) as ep,
            ):
                xfs = ep.tile([128, MC], F32, name="xfs", bufs=1)
                xtfs = ep.tile([128, MC], F32, name="xtfs", bufs=1)
                nc.gpsimd.dma_start(xfs[:], xf[:])
                nc.gpsimd.dma_start(xtfs[:], xtf[:])

                for n in range(NCHK):
                    pg = ps2.tile([128, 512], F32, tag="pg", name=f"pg{n}")
                    for q in range(4):
                        w2t = w2p.tile(
                            [128, KQ * 512], CDT, tag="w2t", name=f"w2t{n}_{q}"
                        )
                        off = (n * 4 + q) * (KQ * 512)
                        eng = nc.sync if q % 2 == 0 else nc.scalar
                        eng.dma_start(w2t[:], w2[:, off : off + KQ * 512])
                        for kk in range(KQ):
                            kg = q * KQ + kk
                            nc.tensor.matmul(
                                pg[:, :],
                                hTf_sb[:, kg * 128 : (kg + 1) * 128],
                                w2t[:, kk * 512 : (kk + 1) * 512],
                                start=(kg == 0),
                                stop=(kg == KT2 - 1),
                            )
                    ms = ep.tile([128, 512], F32, tag="ms", name=f"ms{n}")
                    nc.scalar.activation(
                        ms[:], pg[:, :], mybir.ActivationFunctionType.Sigmoid
                    )
                    us = ep.tile([128, 512], F32, tag="us", name=f"us{n}")
                    nc.vector.tensor_mul(us[:], ms[:], xfs[:, n * 512 : (n + 1) * 512])
                    ls = ep.tile([128, 512], F32, tag="ls", name=f"ls{n}")
                    nc.vector.tensor_mul(ls[:], ms[:], xtfs[:, n * 512 : (n + 1) * 512])
                    for t, dst in ((ms, om), (us, ou), (ls, ol)):
                        nc.scalar.dma_start(dst[:, n * 512 : (n + 1) * 512], t[:])

    nc.compile()
    return nc


def prep_in_maps(x, w1, b1, w2, b2):
    x = np.asarray(x)
    w1 = np.asarray(w1, dtype=np.float32)
    b1 = np.asarray(b1, dtype=np.float32)
    w2 = np.asarray(w2, dtype=np.float32)
    b2 = np.asarray(b2, dtype=np.float32)
    iu0, iu1 = _IU
    xfl = np.ascontiguousarray(x[:, iu0, iu1]).astype(np.float32)   # [B, M]
    xtfl = np.ascontiguousarray(x[:, iu1, iu0]).astype(np.float32)  # [B, M]

    # xT permuted: xT[p, k*128 + b] = x_aug^T[k*128 + p, b]
    xTa = np.zeros((K1, B), dtype=NP_CDT)
    xTa[:M] = xfl.T.astype(NP_CDT)
    xTa[M] = 1.0  # bias-ones row: picks up b1 (and block 7's b2 unit)
    xTp = np.ascontiguousarray(
        xTa.reshape(KT1, 128, B).transpose(1, 0, 2).reshape(128, K1)
    )

    # globally padded column space: 8 blocks of MC; core c owns block c
    MPAD = NCORES * MC
    xf_p = np.zeros((B, MPAD), np.float32)
    xf_p[:, :M] = xfl
    xtf_p = np.zeros((B, MPAD), np.float32)
    xtf_p[:, :M] = xtfl

    # w2 with globally padded hidden rows (8 blocks of HC) and padded cols,
    # b2 folded: block-7 hidden slot BIAS_SLOT is the b2 bias unit.
    w2g = np.zeros((HTOT, MPAD), dtype=NP_CDT)
    for cb in range(NCORES):
        h0, hn = H_START[cb], H_PER[cb]
        w2g[cb * HC : cb * HC + hn, :M] = w2[h0 : h0 + hn, :].astype(NP_CDT)
    w2g[7 * HC + BIAS_SLOT, :M] = b2.astype(NP_CDT)

    in_maps = []
    for c in range(NCORES):
        h0, hn = H_START[c], H_PER[c]
        w1c = np.zeros((K1, HC), dtype=NP_CDT)
        w1c[:M, :hn] = w1[:, h0 : h0 + hn].astype(NP_CDT)
        w1c[M, :hn] = b1[h0 : h0 + hn].astype(NP_CDT)
        if c == NCORES - 1:
            w1c[M, BIAS_SLOT] = 1.0  # h[:, BIAS_SLOT] = relu(1*1) = 1 on core 7 only
        # permute per hidden half: w1p[p, half*KT1*HHALF + k*HHALF + f]
        #   = w1c[k*128 + p, half*HHALF + f]
        halves = []
        for half in range(2):
            blk = w1c[:, half * HHALF : (half + 1) * HHALF]
            halves.append(
                blk.reshape(KT1, 128, HHALF).transpose(1, 0, 2).reshape(128, KT1 * HHALF)
            )
        w1p = np.ascontiguousarray(np.concatenate(halves, axis=1))
        # w2 shard: all hidden rows, own column block; permuted per (n, q):
        # w2p[p, ((n*4+q)*KQ + kk)*512 + f] = w2g[(q*KQ+kk)*128 + p, c*MC + n*512 + f]
        shard = w2g[:, c * MC : (c + 1) * MC].reshape(KT2, 128, MC)
        blocks = []
        for n in range(NCHK):
            for q in range(4):
                blk = shard[q * KQ : (q + 1) * KQ, :, n * 512 : (n + 1) * 512]
                blocks.append(blk.transpose(1, 0, 2).reshape(128, KQ * 512))
        w2p = np.ascontiguousarray(np.concatenate(blocks, axis=1))
        in_maps.append(
            {
                "xT": xTp,
                "w1": w1p,
                "w2": w2p,
                "xf": np.ascontiguousarray(xf_p[:, c * MC : (c + 1) * MC]),
                "xtf": np.ascontiguousarray(xtf_p[:, c * MC : (c + 1) * MC]),
            }
        )
    return in_maps


def assemble(results):
    m = np.concatenate(
        [results[c]["om"][:, : MC_REAL[c]] for c in range(NCORES)], axis=1
    )
    u = np.concatenate(
        [results[c]["ou"][:, : MC_REAL[c]] for c in range(NCORES)], axis=1
    )
    l = np.concatenate(
        [results[c]["ol"][:, : MC_REAL[c]] for c in range(NCORES)], axis=1
    )
    iu0, iu1 = _IU
    out = np.zeros((B, NCH, NCH), np.float32)
    out[:, iu0, iu1] = u
    out[:, iu1, iu0] = l
    return out.astype(np.float32), m.astype(np.float32)


_NC_CACHE = None


def kernel(x, w1, b1, w2, b2, _trace=False):
    global _NC_CACHE
    in_maps = prep_in_maps(x, w1, b1, w2, b2)
    if _NC_CACHE is None:
        _NC_CACHE = build_nc()
    res = bass_utils.run_bass_kernel_spmd(
        _NC_CACHE, in_maps, core_ids=list(range(NCORES)), trace=_trace
    )
    out = assemble(res.results)
    if _trace:
        return out, res
    return out


# revision 19
# speedup vs baseline: 1.0722x; 1.0722x over previous
"""Trainium2 Bass kernel for nn_AdaptiveMask (dense MLP over upper-triangle gather).

Computation (reference):
    x_flat = x[:, iu0, iu1]                      # [B, M] strict-upper-tri gather
    h = relu(x_flat @ w1 + b1)                   # [B, H]
    m = sigmoid(h @ w2 + b2)                     # [B, M]
    W = scatter_sym(m); out = W * x              # [B, C, C]
    returns (out, m)

Strategy (8 NeuronCores):
  - GEMM1 is tensor-parallel over the hidden dim: core c holds w1[:, cols_c]
    and computes h_c = relu(x_flat @ w1_c) for the full batch, in TWO hidden
    halves so the first half's activations AllGather while the second half
    computes (the gather hides under compute).
  - The transposed activations hT ([128, 640] bf16 per half, 0.16 MB) are
    AllGathered so every core holds the full hT; GEMM2 is tensor-parallel
    over the OUTPUT columns — each core computes exact (unsummed)
    y[:, cols_c] = h @ w2[:, cols_c]: no output reduction, no y bounce.
  - All matmuls in bf16 (fp32 PSUM accumulation); epilogue in fp32 straight
    from PSUM (sigmoid on ScalarE, gating multiplies on VectorE).
  - Weights/x are pre-permuted on host into per-partition-contiguous layouts
    so every weight DMA is a plain 2D transfer at line rate.
  - DMA ring discipline: sync + scalar HWDGE rings carry only the weight
    streams (FIFO rings — anything else ahead of weights starves the PE);
    gpsimd (SWDGE) carries xT/xf/xtf, the collectives, and gather loads.
  - Biases are folded into the GEMMs (ones-column in x_flat picks up b1; a
    bias hidden-unit in core 7's hidden block injects b2, shared to all
    cores by the AllGather).
  - Host does only layout (triangle gather/scatter, pad, shard, permute,
    cast); every FLOP of the reference runs on device.
"""

import numpy as np
import ml_dtypes

import concourse.bass as bass
import concourse.bacc as bacc
import concourse.tile as tile
from concourse import mybir
from concourse import bass_utils

# ---- problem constants (must match reference.py's setup_inputs) ----
B, NCH = 128, 200
M, H = 19900, 9950
NCORES = 8
BL = B // NCORES

K1, KT1 = 19968, 156  # GEMM1 contraction (19900 data + 1 bias row + pad), k-tiles
HC = 1280             # per-core hidden slots
HHALF = HC // 2       # hidden half (640)
HTOT, KT2 = 10240, 80 # global padded hidden, GEMM2 k-tiles
H_PER = [1244] * 7 + [1242]
H_START = [0, 1244, 2488, 3732, 4976, 6220, 7464, 8708]
BIAS_SLOT = 1242      # block-7 local hidden slot for the b2 bias unit

MC = 2560             # per-core output columns (5 n-chunks of 512)
MC_REAL = [2560] * 7 + [19900 - 7 * 2560]  # real cols per core (core 7: 1980)
NCHK = 5              # n-chunks per core
KQ = 20               # k-tiles per w2 DMA quarter (4 quarters of 80)

W1_CHUNK = 8          # k-tiles per w1 DMA (1.3 MB bf16 per hidden half)

CDT = mybir.dt.bfloat16
NP_CDT = ml_dtypes.bfloat16
F32 = mybir.dt.float32

_IU = np.triu_indices(NCH, k=1)


def build_nc():
    nc = bacc.Bacc("TRN2", target_bir_lowering=False, debug=False, num_devices=NCORES)

    # host-permuted layouts: per-partition-contiguous (see prep_in_maps)
    xT = nc.dram_tensor("xT", [128, K1], CDT, kind="ExternalInput")
    w1 = nc.dram_tensor("w1", [128, KT1 * HC], CDT, kind="ExternalInput")
    w2 = nc.dram_tensor("w2", [128, KT2 * MC], CDT, kind="ExternalInput")
    xf = nc.dram_tensor("xf", [B, MC], F32, kind="ExternalInput")
    xtf = nc.dram_tensor("xtf", [B, MC], F32, kind="ExternalInput")
    om = nc.dram_tensor("om", [B, MC], F32, kind="ExternalOutput")
    ou = nc.dram_tensor("ou", [B, MC], F32, kind="ExternalOutput")
    ol = nc.dram_tensor("ol", [B, MC], F32, kind="ExternalOutput")

    with tile.TileContext(nc) as tc:
        with (
            tc.tile_pool(name="const", bufs=1) as constp,
            tc.tile_pool(name="hbuf", bufs=1) as hp,
            tc.tile_pool(name="dramc", bufs=1, space="DRAM") as dramc,
        ):
            ident_dram = nc.inline_tensor(np.eye(128, dtype=NP_CDT), name="ident")
            ident = constp.tile([128, 128], CDT)
            nc.scalar.dma_start(ident[:], ident_dram[:])

            # x_flat^T resident in SBUF (gpsimd ring: keeps HWDGE rings clear;
            # two pieces so the first matmuls start after ~2.5 MB)
            xT_sb = constp.tile([128, K1], CDT)
            nc.gpsimd.dma_start(xT_sb[:, : K1 // 2], xT[:, : K1 // 2])
            nc.gpsimd.dma_start(xT_sb[:, K1 // 2 :], xT[:, K1 // 2 :])

            h_sb = hp.tile([128, HC], CDT)      # [batch, hidden_local]
            hT_sb = hp.tile([128, HC], CDT)     # [hidden_local, batch], 10 k-tiles
            hTf_sb = hp.tile([128, HTOT], CDT)  # full gathered hT, 80 k-tiles

            # warm up the collective path while GEMM1 runs: the first ncfw
            # triggers pay a large init cost (measured 132 us cold, ~18 us warm)
            cc_wi = dramc.tile([128, 128], CDT, name="cc_wi")
            for i in range(2):
                cc_wo = dramc.tile(
                    [NCORES * 128, 128], CDT, name=f"cc_wo{i}", addr_space="Shared"
                )
                nc.gpsimd.collective_compute(
                    "AllGather",
                    mybir.AluOpType.bypass,
                    replica_groups=[list(range(NCORES))],
                    ins=[cc_wi[:].opt()],
                    outs=[cc_wo[:].opt()],
                )

            hT_bounce = [
                dramc.tile([128, HHALF], CDT, name=f"hT_bounce{h}") for h in range(2)
            ]
            hT_all = [
                dramc.tile(
                    [NCORES * 128, HHALF], CDT, name=f"hT_all{h}", addr_space="Shared"
                )
                for h in range(2)
            ]

            # ---- GEMM1: h = relu(x_flat @ w1_c + b1_c), two hidden halves ----
            with (
                tc.tile_pool(name="w1p", bufs=4) as w1p,
                tc.tile_pool(name="ps1", bufs=2, space="PSUM") as ps1,
                tc.tile_pool(name="pst", bufs=2, space="PSUM") as pst,
            ):
                n_chunks1 = [(0, 512), (512, 640)]
                n_w1_chunks = (KT1 + W1_CHUNK - 1) // W1_CHUNK
                for half in range(2):
                    ph = [
                        ps1.tile([128, n1 - n0], F32, tag=f"ph{i}", name=f"ph{i}_{half}")
                        for i, (n0, n1) in enumerate(n_chunks1)
                    ]
                    hbase = half * KT1 * HHALF
                    for c in range(n_w1_chunks):
                        kc = min(W1_CHUNK, KT1 - c * W1_CHUNK)
                        c0 = hbase + c * W1_CHUNK * HHALF
                        w1t = w1p.tile(
                            [128, kc * HHALF], CDT, tag="w1t", name=f"w1t{half}_{c}"
                        )
                        eng = nc.sync if c % 2 == 0 else nc.scalar
                        eng.dma_start(w1t[:], w1[:, c0 : c0 + kc * HHALF])
                        for j in range(kc):
                            k = c * W1_CHUNK + j
                            for i, (n0, n1) in enumerate(n_chunks1):
                                nc.tensor.matmul(
                                    ph[i][:, :],
                                    xT_sb[:, k * 128 : (k + 1) * 128],
                                    w1t[:, j * HHALF + n0 : j * HHALF + n1],
                                    start=(k == 0),
                                    stop=(k == KT1 - 1),
                                )
                    for i, (n0, n1) in enumerate(n_chunks1):
                        nc.scalar.activation(
                            h_sb[:, half * HHALF + n0 : half * HHALF + n1],
                            ph[i][:, :],
                            mybir.ActivationFunctionType.Relu,
                        )
                    # transpose this half's h -> hT via PE
                    for j in range(HHALF // 128):
                        pt = pst.tile([128, 128], CDT, tag="pt", name=f"pt{half}_{j}")
                        nc.tensor.transpose(
                            pt[:],
                            h_sb[:, half * HHALF + j * 128 : half * HHALF + (j + 1) * 128],
                            ident[:],
                        )
                        nc.vector.tensor_copy(
                            hT_sb[:, half * HHALF + j * 128 : half * HHALF + (j + 1) * 128],
                            pt[:],
                        )
                    # AllGather this half (first half hides under second half's GEMM)
                    nc.gpsimd.dma_start(
                        hT_bounce[half][:], hT_sb[:, half * HHALF : (half + 1) * HHALF]
                    )
                    nc.gpsimd.collective_compute(
                        "AllGather",
                        mybir.AluOpType.bypass,
                        replica_groups=[list(range(NCORES))],
                        ins=[hT_bounce[half][:].opt()],
                        outs=[hT_all[half][:].opt()],
                    )
                    # pass-major layout: hTf free = (half, core, k_local, batch)
                    for c in range(NCORES):
                        nc.gpsimd.dma_start(
                            hTf_sb[
                                :,
                                half * (NCORES * HHALF)
                                + c * HHALF : half * (NCORES * HHALF)
                                + (c + 1) * HHALF,
                            ],
                            hT_all[half][c * 128 : (c + 1) * 128, :],
                        )

            # ---- GEMM2 (exact column shard, no reduce) + fused epilogue ----
            with (
                tc.tile_pool(name="w2p", bufs=4) as w2p,
                tc.tile_pool(name="ps2", bufs=1, space="PSUM") as ps2,
                tc.tile_pool(name="ep", bufs=2) as ep,
            ):
                xfs = ep.tile([128, MC], F32, name="xfs", bufs=1)
                xtfs = ep.tile([128, MC], F32, name="xtfs", bufs=1)
                nc.gpsimd.dma_start(xfs[:], xf[:])
                nc.gpsimd.dma_start(xtfs[:], xtf[:])

                # two k-passes over the hidden halves: pass 0 needs only the
                # first AllGather (hidden under GEMM1 half 1); the second
                # AllGather hides under pass 0's ~75 us of work. The five
                # per-n-chunk PSUM banks accumulate across both passes.
                pgs = [
                    ps2.tile([128, 512], F32, tag=f"pg{n}", name=f"pg{n}")
                    for n in range(NCHK)
                ]
                for ps in range(2):
                    for n in range(NCHK):
                        pg = pgs[n]
                        for q in range(2):
                            w2t = w2p.tile(
                                [128, KQ * 512], CDT, tag="w2t", name=f"w2t{ps}_{n}_{q}"
                            )
                            off = ((ps * NCHK + n) * 2 + q) * (KQ * 512)
                            eng = nc.sync if q % 2 == 0 else nc.scalar
                            eng.dma_start(w2t[:], w2[:, off : off + KQ * 512])
                            for kk in range(KQ):
                                kg = (ps * 2 + q) * KQ + kk
                                nc.tensor.matmul(
                                    pg[:, :],
                                    hTf_sb[:, kg * 128 : (kg + 1) * 128],
                                    w2t[:, kk * 512 : (kk + 1) * 512],
                                    start=(ps == 0 and q == 0 and kk == 0),
                                    stop=(ps == 1 and q == 1 and kk == KQ - 1),
                                )
                        if ps == 1:
                            ms = ep.tile([128, 512], F32, tag="ms", name=f"ms{n}")
                            nc.scalar.activation(
                                ms[:], pg[:, :], mybir.ActivationFunctionType.Sigmoid
                            )
                            us = ep.tile([128, 512], F32, tag="us", name=f"us{n}")
                            nc.vector.tensor_mul(
                                us[:], ms[:], xfs[:, n * 512 : (n + 1) * 512]
                            )
                            ls = ep.tile([128, 512], F32, tag="ls", name=f"ls{n}")
                            nc.vector.tensor_mul(
                                ls[:], ms[:], xtfs[:, n * 512 : (n + 1) * 512]
                            )
                            for t, dst in ((ms, om), (us, ou), (ls, ol)):
                                nc.gpsimd.dma_start(
                                    dst[:, n * 512 : (n + 1) * 512], t[:]
                                )

    nc.compile()
    return nc


def prep_in_maps(x, w1, b1, w2, b2):
    x = np.asarray(x)
    w1 = np.asarray(w1, dtype=np.float32)
    b1 = np.asarray(b1, dtype=np.float32)
    w2 = np.asarray(w2, dtype=np.float32)
    b2 = np.asarray(b2, dtype=np.float32)
    iu0, iu1 = _IU
    xfl = np.ascontiguousarray(x[:, iu0, iu1]).astype(np.float32)   # [B, M]
    xtfl = np.ascontiguousarray(x[:, iu1, iu0]).astype(np.float32)  # [B, M]

    # xT permuted: xT[p, k*128 + b] = x_aug^T[k*128 + p, b]
    xTa = np.zeros((K1, B), dtype=NP_CDT)
    xTa[:M] = xfl.T.astype(NP_CDT)
    xTa[M] = 1.0  # bias-ones row: picks up b1 (and block 7's b2 unit)
    xTp = np.ascontiguousarray(
        xTa.reshape(KT1, 128, B).transpose(1, 0, 2).reshape(128, K1)
    )

    # globally padded column space: 8 blocks of MC; core c owns block c
    MPAD = NCORES * MC
    xf_p = np.zeros((B, MPAD), np.float32)
    xf_p[:, :M] = xfl
    xtf_p = np.zeros((B, MPAD), np.float32)
    xtf_p[:, :M] = xtfl

    # w2 with globally padded hidden rows (8 blocks of HC) and padded cols,
    # b2 folded: block-7 hidden slot BIAS_SLOT is the b2 bias unit.
    w2g = np.zeros((HTOT, MPAD), dtype=NP_CDT)
    for cb in range(NCORES):
        h0, hn = H_START[cb], H_PER[cb]
        w2g[cb * HC : cb * HC + hn, :M] = w2[h0 : h0 + hn, :].astype(NP_CDT)
    w2g[7 * HC + BIAS_SLOT, :M] = b2.astype(NP_CDT)

    in_maps = []
    for c in range(NCORES):
        h0, hn = H_START[c], H_PER[c]
        w1c = np.zeros((K1, HC), dtype=NP_CDT)
        w1c[:M, :hn] = w1[:, h0 : h0 + hn].astype(NP_CDT)
        w1c[M, :hn] = b1[h0 : h0 + hn].astype(NP_CDT)
        if c == NCORES - 1:
            w1c[M, BIAS_SLOT] = 1.0  # h[:, BIAS_SLOT] = relu(1*1) = 1 on core 7 only
        # permute per hidden half: w1p[p, half*KT1*HHALF + k*HHALF + f]
        #   = w1c[k*128 + p, half*HHALF + f]
        halves = []
        for half in range(2):
            blk = w1c[:, half * HHALF : (half + 1) * HHALF]
            halves.append(
                blk.reshape(KT1, 128, HHALF).transpose(1, 0, 2).reshape(128, KT1 * HHALF)
            )
        w1p = np.ascontiguousarray(np.concatenate(halves, axis=1))
        # w2 shard: all hidden rows reordered PASS-major (half, core, slot),
        # own column block; permuted into DMA blocks of (pass, n, q):
        # k-tile kt = pass*40 + q*KQ + kk maps to rows (half=pass, cb, kl)
        w2r = np.zeros((HTOT, MC), dtype=NP_CDT)
        shard_cols = w2g[:, c * MC : (c + 1) * MC]
        for half in range(2):
            for cb in range(NCORES):
                w2r[
                    half * (NCORES * HHALF)
                    + cb * HHALF : half * (NCORES * HHALF)
                    + (cb + 1) * HHALF
                ] = shard_cols[cb * HC + half * HHALF : cb * HC + (half + 1) * HHALF]
        shard = w2r.reshape(KT2, 128, MC)
        blocks = []
        for ps in range(2):
            for n in range(NCHK):
                for q in range(2):
                    k0 = ps * 40 + q * KQ
                    blk = shard[k0 : k0 + KQ, :, n * 512 : (n + 1) * 512]
                    blocks.append(blk.transpose(1, 0, 2).reshape(128, KQ * 512))
        w2p = np.ascontiguousarray(np.concatenate(blocks, axis=1))
        in_maps.append(
            {
                "xT": xTp,
                "w1": w1p,
                "w2": w2p,
                "xf": np.ascontiguousarray(xf_p[:, c * MC : (c + 1) * MC]),
                "xtf": np.ascontiguousarray(xtf_p[:, c * MC : (c + 1) * MC]),
            }
        )
    return in_maps


def assemble(results):
    m = np.concatenate(
        [results[c]["om"][:, : MC_REAL[c]] for c in range(NCORES)], axis=1
    )
    u = np.concatenate(
        [results[c]["ou"][:, : MC_REAL[c]] for c in range(NCORES)], axis=1
    )
    l = np.concatenate(
        [results[c]["ol"][:, : MC_REAL[c]] for c in range(NCORES)], axis=1
    )
    iu0, iu1 = _IU
    out = np.zeros((B, NCH, NCH), np.float32)
    out[:, iu0, iu1] = u
    out[:, iu1, iu0] = l
    return out.astype(np.float32), m.astype(np.float32)


_NC_CACHE = None


def kernel(x, w1, b1, w2, b2, _trace=False):
    global _NC_CACHE
    in_maps = prep_in_maps(x, w1, b1, w2, b2)
    if _NC_CACHE is None:
        _NC_CACHE = build_nc()
    res = bass_utils.run_bass_kernel_spmd(
        _NC_CACHE, in_maps, core_ids=list(range(NCORES)), trace=_trace
    )
    out = assemble(res.results)
    if _trace:
        return out, res
    return out


# revision 20
# speedup vs baseline: 1.0839x; 1.0109x over previous
"""Trainium2 Bass kernel for nn_AdaptiveMask (dense MLP over upper-triangle gather).

Computation (reference):
    x_flat = x[:, iu0, iu1]                      # [B, M] strict-upper-tri gather
    h = relu(x_flat @ w1 + b1)                   # [B, H]
    m = sigmoid(h @ w2 + b2)                     # [B, M]
    W = scatter_sym(m); out = W * x              # [B, C, C]
    returns (out, m)

Strategy (8 NeuronCores):
  - GEMM1 is tensor-parallel over the hidden dim: core c holds w1[:, cols_c]
    and computes h_c = relu(x_flat @ w1_c) for the full batch, in two uneven
    hidden pieces (768 + 512 slots) so each piece's activations AllGather
    while later compute runs (both collectives fully hidden).
  - The transposed activations hT (0.2/0.13 MB bf16 per piece) are
    AllGathered so every core holds the full hT; GEMM2 is tensor-parallel
    over the OUTPUT columns — each core computes exact (unsummed)
    y[:, cols_c] = h @ w2[:, cols_c] in two k-passes (one per hidden piece):
    pass 0 needs only the first AllGather; the second AllGather hides under
    pass 0's ~100 us of DMA-bound work. No output reduction, no y bounce.
  - All matmuls in bf16 (fp32 PSUM accumulation); epilogue in fp32 straight
    from PSUM (sigmoid on ScalarE, gating multiplies on VectorE).
  - Weights/x are pre-permuted on host into per-partition-contiguous layouts
    so every weight DMA is a plain 2D transfer at line rate.
  - DMA ring discipline: sync + scalar HWDGE rings carry only the weight
    streams (FIFO rings — anything else ahead of weights starves the PE);
    gpsimd (SWDGE) carries xT/xf/xtf, the collectives, gather loads, and
    the output stores.
  - Biases are folded into the GEMMs (ones-column in x_flat picks up b1; a
    bias hidden-unit in core 7's hidden block injects b2, shared to all
    cores by the AllGather).
  - Host does only layout (triangle gather/scatter, pad, shard, permute,
    cast); every FLOP of the reference runs on device.
"""

import numpy as np
import ml_dtypes

import concourse.bass as bass
import concourse.bacc as bacc
import concourse.tile as tile
from concourse import mybir
from concourse import bass_utils

# ---- problem constants (must match reference.py's setup_inputs) ----
B, NCH = 128, 200
M, H = 19900, 9950
NCORES = 8
BL = B // NCORES

K1, KT1 = 19968, 156  # GEMM1 contraction (19900 data + 1 bias row + pad), k-tiles
HC = 1280             # per-core hidden slots
H_SPLIT = [768, 512]  # uneven hidden pieces (AllGather hiding margins)
H_OFF = [0, 768]
HTOT, KT2 = 10240, 80 # global padded hidden, GEMM2 k-tiles
H_PER = [1244] * 7 + [1242]
H_START = [0, 1244, 2488, 3732, 4976, 6220, 7464, 8708]
BIAS_SLOT = 1242      # block-7 local hidden slot for the b2 bias unit

MC = 2560             # per-core output columns (5 n-chunks of 512)
MC_REAL = [2560] * 7 + [19900 - 7 * 2560]  # real cols per core (core 7: 1980)
NCHK = 5              # n-chunks per core
KQ = 16               # k-tiles per w2 DMA block (2.1 MB bf16)
# GEMM2 k-tiles per pass and DMA blocks per (pass, n)
PASS_KT = [NCORES * H_SPLIT[0] // 128, NCORES * H_SPLIT[1] // 128]  # [48, 32]
PASS_NQ = [PASS_KT[0] // KQ, PASS_KT[1] // KQ]                      # [3, 2]

W1_CHUNK = 8          # k-tiles per w1 DMA chunk

CDT = mybir.dt.bfloat16
NP_CDT = ml_dtypes.bfloat16
F32 = mybir.dt.float32

_IU = np.triu_indices(NCH, k=1)


def build_nc():
    nc = bacc.Bacc("TRN2", target_bir_lowering=False, debug=False, num_devices=NCORES)

    # host-permuted layouts: per-partition-contiguous (see prep_in_maps)
    xT = nc.dram_tensor("xT", [128, K1], CDT, kind="ExternalInput")
    w1 = nc.dram_tensor("w1", [128, KT1 * HC], CDT, kind="ExternalInput")
    w2 = nc.dram_tensor("w2", [128, KT2 * MC], CDT, kind="ExternalInput")
    xf = nc.dram_tensor("xf", [B, MC], F32, kind="ExternalInput")
    xtf = nc.dram_tensor("xtf", [B, MC], F32, kind="ExternalInput")
    om = nc.dram_tensor("om", [B, MC], F32, kind="ExternalOutput")
    ou = nc.dram_tensor("ou", [B, MC], F32, kind="ExternalOutput")
    ol = nc.dram_tensor("ol", [B, MC], F32, kind="ExternalOutput")

    with tile.TileContext(nc) as tc:
        with (
            tc.tile_pool(name="const", bufs=1) as constp,
            tc.tile_pool(name="hbuf", bufs=1) as hp,
            tc.tile_pool(name="dramc", bufs=1, space="DRAM") as dramc,
        ):
            ident_dram = nc.inline_tensor(np.eye(128, dtype=NP_CDT), name="ident")
            ident = constp.tile([128, 128], CDT)
            nc.scalar.dma_start(ident[:], ident_dram[:])

            # x_flat^T resident in SBUF (gpsimd ring: keeps HWDGE rings clear;
            # two pieces so the first matmuls start after ~2.5 MB)
            xT_sb = constp.tile([128, K1], CDT)
            nc.gpsimd.dma_start(xT_sb[:, : K1 // 2], xT[:, : K1 // 2])
            nc.gpsimd.dma_start(xT_sb[:, K1 // 2 :], xT[:, K1 // 2 :])

            h_sb = hp.tile([128, HC], CDT)      # [batch, hidden_local]
            hT_sb = hp.tile([128, HC], CDT)     # [hidden_local, batch], 10 k-tiles
            hTf_sb = hp.tile([128, HTOT], CDT)  # full gathered hT, pass-major

            # warm up the collective path while GEMM1 runs: the first ncfw
            # triggers pay a large init cost (measured 132 us cold, ~18 us warm)
            cc_wi = dramc.tile([128, 128], CDT, name="cc_wi")
            for i in range(2):
                cc_wo = dramc.tile(
                    [NCORES * 128, 128], CDT, name=f"cc_wo{i}", addr_space="Shared"
                )
                nc.gpsimd.collective_compute(
                    "AllGather",
                    mybir.AluOpType.bypass,
                    replica_groups=[list(range(NCORES))],
                    ins=[cc_wi[:].opt()],
                    outs=[cc_wo[:].opt()],
                )

            hT_bounce = [
                dramc.tile([128, H_SPLIT[h]], CDT, name=f"hT_bounce{h}")
                for h in range(2)
            ]
            hT_all = [
                dramc.tile(
                    [NCORES * 128, H_SPLIT[h]], CDT, name=f"hT_all{h}",
                    addr_space="Shared",
                )
                for h in range(2)
            ]

            # ---- GEMM1: h = relu(x_flat @ w1_c + b1_c), two hidden pieces ----
            with (
                tc.tile_pool(name="w1p", bufs=4) as w1p,
                tc.tile_pool(name="ps1", bufs=2, space="PSUM") as ps1,
                tc.tile_pool(name="pst", bufs=2, space="PSUM") as pst,
            ):
                n_w1_chunks = (KT1 + W1_CHUNK - 1) // W1_CHUNK
                w1_hbase = 0
                for half in range(2):
                    hw = H_SPLIT[half]
                    n_chunks1 = (
                        [(0, 512), (512, 768)] if half == 0 else [(0, 512)]
                    )
                    ph = [
                        ps1.tile([128, n1 - n0], F32, tag=f"ph{i}", name=f"ph{i}_{half}")
                        for i, (n0, n1) in enumerate(n_chunks1)
                    ]
                    for c in range(n_w1_chunks):
                        kc = min(W1_CHUNK, KT1 - c * W1_CHUNK)
                        c0 = w1_hbase + c * W1_CHUNK * hw
                        w1t = w1p.tile(
                            [128, kc * hw], CDT, tag="w1t", name=f"w1t{half}_{c}"
                        )
                        eng = nc.sync if c % 2 == 0 else nc.scalar
                        eng.dma_start(w1t[:], w1[:, c0 : c0 + kc * hw])
                        for j in range(kc):
                            k = c * W1_CHUNK + j
                            for i, (n0, n1) in enumerate(n_chunks1):
                                nc.tensor.matmul(
                                    ph[i][:, :],
                                    xT_sb[:, k * 128 : (k + 1) * 128],
                                    w1t[:, j * hw + n0 : j * hw + n1],
                                    start=(k == 0),
                                    stop=(k == KT1 - 1),
                                )
                    w1_hbase += KT1 * hw
                    for i, (n0, n1) in enumerate(n_chunks1):
                        nc.scalar.activation(
                            h_sb[:, H_OFF[half] + n0 : H_OFF[half] + n1],
                            ph[i][:, :],
                            mybir.ActivationFunctionType.Relu,
                        )
                    # transpose this piece's h -> hT via PE
                    for j in range(hw // 128):
                        pt = pst.tile([128, 128], CDT, tag="pt", name=f"pt{half}_{j}")
                        nc.tensor.transpose(
                            pt[:],
                            h_sb[:, H_OFF[half] + j * 128 : H_OFF[half] + (j + 1) * 128],
                            ident[:],
                        )
                        nc.vector.tensor_copy(
                            hT_sb[:, H_OFF[half] + j * 128 : H_OFF[half] + (j + 1) * 128],
                            pt[:],
                        )
                    # AllGather this piece (hidden under later compute)
                    nc.gpsimd.dma_start(
                        hT_bounce[half][:],
                        hT_sb[:, H_OFF[half] : H_OFF[half] + hw],
                    )
                    nc.gpsimd.collective_compute(
                        "AllGather",
                        mybir.AluOpType.bypass,
                        replica_groups=[list(range(NCORES))],
                        ins=[hT_bounce[half][:].opt()],
                        outs=[hT_all[half][:].opt()],
                    )
                    # pass-major layout: hTf free = (piece, core, k_local, batch)
                    pbase = half * NCORES * H_SPLIT[0]
                    for c in range(NCORES):
                        nc.gpsimd.dma_start(
                            hTf_sb[:, pbase + c * hw : pbase + (c + 1) * hw],
                            hT_all[half][c * 128 : (c + 1) * 128, :],
                        )

            # ---- GEMM2 (exact column shard, no reduce) + fused epilogue ----
            with (
                tc.tile_pool(name="w2p", bufs=4) as w2p,
                tc.tile_pool(name="ps2", bufs=1, space="PSUM") as ps2,
                tc.tile_pool(name="ep", bufs=2) as ep,
            ):
                xfs = ep.tile([128, MC], F32, name="xfs", bufs=1)
                xtfs = ep.tile([128, MC], F32, name="xtfs", bufs=1)
                nc.gpsimd.dma_start(xfs[:], xf[:])
                nc.gpsimd.dma_start(xtfs[:], xtf[:])

                # two k-passes over the hidden pieces; five per-n-chunk PSUM
                # banks accumulate across both passes
                pgs = [
                    ps2.tile([128, 512], F32, tag=f"pg{n}", name=f"pg{n}")
                    for n in range(NCHK)
                ]
                blk_off = 0
                for ps in range(2):
                    kt_base = PASS_KT[0] if ps == 1 else 0
                    for n in range(NCHK):
                        pg = pgs[n]
                        for q in range(PASS_NQ[ps]):
                            w2t = w2p.tile(
                                [128, KQ * 512], CDT, tag="w2t", name=f"w2t{ps}_{n}_{q}"
                            )
                            off = blk_off + (n * PASS_NQ[ps] + q) * (KQ * 512)
                            eng = nc.sync if q % 2 == 0 else nc.scalar
                            eng.dma_start(w2t[:], w2[:, off : off + KQ * 512])
                            for kk in range(KQ):
                                kg = kt_base + q * KQ + kk
                                nc.tensor.matmul(
                                    pg[:, :],
                                    hTf_sb[:, kg * 128 : (kg + 1) * 128],
                                    w2t[:, kk * 512 : (kk + 1) * 512],
                                    start=(ps == 0 and q == 0 and kk == 0),
                                    stop=(
                                        ps == 1
                                        and q == PASS_NQ[1] - 1
                                        and kk == KQ - 1
                                    ),
                                )
                        if ps == 1:
                            ms = ep.tile([128, 512], F32, tag="ms", name=f"ms{n}")
                            nc.scalar.activation(
                                ms[:], pg[:, :], mybir.ActivationFunctionType.Sigmoid
                            )
                            us = ep.tile([128, 512], F32, tag="us", name=f"us{n}")
                            nc.vector.tensor_mul(
                                us[:], ms[:], xfs[:, n * 512 : (n + 1) * 512]
                            )
                            ls = ep.tile([128, 512], F32, tag="ls", name=f"ls{n}")
                            nc.vector.tensor_mul(
                                ls[:], ms[:], xtfs[:, n * 512 : (n + 1) * 512]
                            )
                            for t, dst in ((ms, om), (us, ou), (ls, ol)):
                                nc.gpsimd.dma_start(
                                    dst[:, n * 512 : (n + 1) * 512], t[:]
                                )
                    blk_off += PASS_KT[ps] * NCHK * 512

    nc.compile()
    return nc


def prep_in_maps(x, w1, b1, w2, b2):
    x = np.asarray(x)
    w1 = np.asarray(w1, dtype=np.float32)
    b1 = np.asarray(b1, dtype=np.float32)
    w2 = np.asarray(w2, dtype=np.float32)
    b2 = np.asarray(b2, dtype=np.float32)
    iu0, iu1 = _IU
    xfl = np.ascontiguousarray(x[:, iu0, iu1]).astype(np.float32)   # [B, M]
    xtfl = np.ascontiguousarray(x[:, iu1, iu0]).astype(np.float32)  # [B, M]

    # xT permuted: xT[p, k*128 + b] = x_aug^T[k*128 + p, b]
    xTa = np.zeros((K1, B), dtype=NP_CDT)
    xTa[:M] = xfl.T.astype(NP_CDT)
    xTa[M] = 1.0  # bias-ones row: picks up b1 (and block 7's b2 unit)
    xTp = np.ascontiguousarray(
        xTa.reshape(KT1, 128, B).transpose(1, 0, 2).reshape(128, K1)
    )

    # globally padded column space: 8 blocks of MC; core c owns block c
    MPAD = NCORES * MC
    xf_p = np.zeros((B, MPAD), np.float32)
    xf_p[:, :M] = xfl
    xtf_p = np.zeros((B, MPAD), np.float32)
    xtf_p[:, :M] = xtfl

    # w2 with globally padded hidden rows (8 blocks of HC) and padded cols,
    # b2 folded: block-7 hidden slot BIAS_SLOT is the b2 bias unit.
    w2g = np.zeros((HTOT, MPAD), dtype=NP_CDT)
    for cb in range(NCORES):
        h0, hn = H_START[cb], H_PER[cb]
        w2g[cb * HC : cb * HC + hn, :M] = w2[h0 : h0 + hn, :].astype(NP_CDT)
    w2g[7 * HC + BIAS_SLOT, :M] = b2.astype(NP_CDT)

    in_maps = []
    for c in range(NCORES):
        h0, hn = H_START[c], H_PER[c]
        w1c = np.zeros((K1, HC), dtype=NP_CDT)
        w1c[:M, :hn] = w1[:, h0 : h0 + hn].astype(NP_CDT)
        w1c[M, :hn] = b1[h0 : h0 + hn].astype(NP_CDT)
        if c == NCORES - 1:
            w1c[M, BIAS_SLOT] = 1.0  # h[:, BIAS_SLOT] = relu(1*1) = 1 on core 7 only
        # permute per hidden piece: w1p[p, base + k*hw + f] = w1c[k*128+p, H_OFF+f]
        pieces = []
        for half in range(2):
            hw = H_SPLIT[half]
            blk = w1c[:, H_OFF[half] : H_OFF[half] + hw]
            pieces.append(
                blk.reshape(KT1, 128, hw).transpose(1, 0, 2).reshape(128, KT1 * hw)
            )
        w1p = np.ascontiguousarray(np.concatenate(pieces, axis=1))
        # w2 shard: hidden rows reordered PASS-major (piece, core, slot), own
        # column block; permuted into DMA blocks of (pass, n, q)
        w2r = np.zeros((HTOT, MC), dtype=NP_CDT)
        shard_cols = w2g[:, c * MC : (c + 1) * MC]
        row = 0
        for half in range(2):
            hw = H_SPLIT[half]
            for cb in range(NCORES):
                w2r[row : row + hw] = shard_cols[
                    cb * HC + H_OFF[half] : cb * HC + H_OFF[half] + hw
                ]
                row += hw
        shard = w2r.reshape(KT2, 128, MC)
        blocks = []
        for ps in range(2):
            kt_base = PASS_KT[0] if ps == 1 else 0
            for n in range(NCHK):
                for q in range(PASS_NQ[ps]):
                    k0 = kt_base + q * KQ
                    blk = shard[k0 : k0 + KQ, :, n * 512 : (n + 1) * 512]
                    blocks.append(blk.transpose(1, 0, 2).reshape(128, KQ * 512))
        w2p = np.ascontiguousarray(np.concatenate(blocks, axis=1))
        in_maps.append(
            {
                "xT": xTp,
                "w1": w1p,
                "w2": w2p,
                "xf": np.ascontiguousarray(xf_p[:, c * MC : (c + 1) * MC]),
                "xtf": np.ascontiguousarray(xtf_p[:, c * MC : (c + 1) * MC]),
            }
        )
    return in_maps


def assemble(results):
    m = np.concatenate(
        [results[c]["om"][:, : MC_REAL[c]] for c in range(NCORES)], axis=1
    )
    u = np.concatenate(
        [results[c]["ou"][:, : MC_REAL[c]] for c in range(NCORES)], axis=1
    )
    l = np.concatenate(
        [results[c]["ol"][:, : MC_REAL[c]] for c in range(NCORES)], axis=1
    )
    iu0, iu1 = _IU
    out = np.zeros((B, NCH, NCH), np.float32)
    out[:, iu0, iu1] = u
    out[:, iu1, iu0] = l
    return out.astype(np.float32), m.astype(np.float32)


_NC_CACHE = None


def kernel(x, w1, b1, w2, b2, _trace=False):
    global _NC_CACHE
    in_maps = prep_in_maps(x, w1, b1, w2, b2)
    if _NC_CACHE is None:
        _NC_CACHE = build_nc()
    res = bass_utils.run_bass_kernel_spmd(
        _NC_CACHE, in_maps, core_ids=list(range(NCORES)), trace=_trace
    )
    out = assemble(res.results)
    if _trace:
        return out, res
    return out
